# revision 20
# baseline (speedup 1.0000x reference)
"""Trainium2 Bass kernel for nn_MultiHeadAttention_60851096649901.

Sharding: 8 cores = 4 batches x 2 head-groups (8 heads each).
Each core computes its batch's attention for its 8 heads plus the partial
out-projection; host sums the two head-group partials and adds bo.

v4 structure (823us baseline -> 553us v2 -> this):
 - pv PSUM double-buffered + normalization deferred into the filler stream
   (killed the HAM re-throttle that kept the PE at 1.2 GHz 68% of the
   time); reciprocal_approx_fast (input must be at partition 0!) + GpSimd
   partition_broadcast.
 - fp16 score path (x, Wq/Wk, qT/kT): FWL weight loads, half DMA. bf16
   q/k fails the 2e-2 tolerance (exp(8s) amplifies); et/v stay f32r (bf16
   ACT output measured +20% slower; f32r cannot col-tile so the M=65
   ones-column PV is the fastest correct form).
 - V projection N=512 (ones columns memset once, not projected).
 - out-projection accumulated across head-pairs in PSUM at the tail: one
   [S,E] bf16 output per core.

Per-core math:
  qT/kT = (Wg.T @ x.T + b)        [128, 2048] per pair (d-major, fp16)
  v     = x @ Wv + bv             [2048, 8, 65] f32r (col 64 = 1.0)
  per (pair, q-chunk 512, k-tile 128):
    scoresT[k, qA|qB] via two concurrent row-group matmuls (K=64, fp16)
    p = exp(8*scores - 100) in ONE [128,1024] ACT instr -> et f32r
    pv_h[65, q] += v_h.T @ p_h  (row 64 = softmax denominator)
  normalization (deferred filler): outhT = pv[0:64] * bcast(recip(pv[64]))
  tail: y[q, :] = sum_pairs outhT_pair.T @ Wo_pair  (PSUM-accumulated)
"""

import numpy as np

S = 2048
E = 1024
D = 64
P = 128
HCORE = 8          # heads per core
NPAIR = 4          # head-pairs per core
C_OFF = 100.0      # softmax constant offset (exp(8*s - C))
INV_SCALE = 8.0    # sqrt(head_dim)

_BUILT = None


def _build():
    import concourse.bass as bass
    import concourse.tile as tile
    from concourse import bacc, mybir

    f32 = mybir.dt.float32
    f32r = mybir.dt.float32r
    bf16 = mybir.dt.bfloat16
    f16 = mybir.dt.float16
    Exp = mybir.ActivationFunctionType.Exp

    nc = bacc.Bacc("TRN2", target_bir_lowering=False, debug=False, num_devices=8)

    xT_d = nc.dram_tensor("xT", [E, S], f16, kind="ExternalInput")
    wq_d = nc.dram_tensor("wq", [E, 512], f16, kind="ExternalInput")
    wk_d = nc.dram_tensor("wk", [E, 512], f16, kind="ExternalInput")
    bq_d = nc.dram_tensor("bq", [4, P, 1], f32, kind="ExternalInput")
    bk_d = nc.dram_tensor("bk", [4, P, 1], f32, kind="ExternalInput")
    wv_d = nc.dram_tensor("wv", [E, 512], f16, kind="ExternalInput")
    bv_d = nc.dram_tensor("bv", [1, 512], f32, kind="ExternalInput")
    wo_d = nc.dram_tensor("wo", [512, E], bf16, kind="ExternalInput")
    y_d = nc.dram_tensor("y", [S, E], bf16, kind="ExternalOutput")

    with tile.TileContext(nc) as tc:
        with (
            tc.tile_pool(name="persist", bufs=1) as persist,
            tc.tile_pool(name="wpool", bufs=2) as wpool,
            tc.tile_pool(name="wopool", bufs=4) as wopool,
            tc.tile_pool(name="qk", bufs=2) as qkpool,
            tc.tile_pool(name="att", bufs=4) as att,
            tc.tile_pool(name="norm", bufs=3) as norm,
            tc.tile_pool(name="oh", bufs=4) as ohpool,
            tc.tile_pool(name="yout", bufs=2) as yout,
            tc.tile_pool(name="ps", bufs=2, space="PSUM") as ps,      # scores+fill
            tc.tile_pool(name="pvps", bufs=2, space="PSUM") as pvps,  # pv + den
        ):
            # ---- persistent loads -------------------------------------------
            xT = persist.tile([P, 8, S], f16, tag="xT")  # [i-part, i-tile, q]
            # v layout [tok-part, kt, head, 65]: col 64 of each head is the
            # constant ones column (softmax denominator via the PV matmul);
            # memset once, the V projection only writes cols 0-63
            v_sb = persist.tile([P, 16, HCORE, 65], f32r, tag="v_sb")
            for h in range(HCORE):
                nc.vector.memset(v_sb[:, :, h, 64:65].bitcast(f32), 1.0)

            neg_c = persist.tile([P, 1], f32, tag="neg_c")
            nc.vector.memset(neg_c[:], -C_OFF)
            # warm up the exp table-set (~2.7us) during the preamble
            warm = persist.tile([P, 1], f32, tag="warm")
            nc.scalar.activation(out=warm[:], in_=neg_c[:], func=Exp,
                                 bias=neg_c[:], scale=1.0)

            bvf = persist.tile([1, 512], f32, tag="bvf")
            nc.sync.dma_start(bvf[:], bv_d[:])
            bvb = persist.tile([P, 512], f32, tag="bvb")
            nc.gpsimd.partition_broadcast(bvb[:], bvf[:], channels=P)

            wv = persist.tile([P, 8, 512], f16, tag="wv")
            # interleave wv with the first xT chunk so the V projection can
            # start as soon as both land
            for i in range(8):
                nc.sync.dma_start(
                    xT[:, i, 0:512], xT_d[i * P:(i + 1) * P, 0:512])
                nc.sync.dma_start(wv[:, i, :], wv_d[i * P:(i + 1) * P, :])
            for cc in range(1, 4):
                for i in range(8):
                    nc.sync.dma_start(
                        xT[:, i, cc * 512:(cc + 1) * 512],
                        xT_d[i * P:(i + 1) * P, cc * 512:(cc + 1) * 512],
                    )

            # ---- op generators (emitted upfront or as attention fillers) ----
            def v_chunk_ops(kt, pool):
                """v[:, kt, :, 0:64] = x @ Wv + bv (9 MMs + evict)."""
                st = {}
                ops = []

                def mk_mm(i):
                    def op():
                        if i == 0:
                            st["p"] = pool.tile([P, 512], f32, tag="sc",
                                                name=f"vps{kt}")
                        nc.tensor.matmul(
                            st["p"][:], xT[:, i, kt * P:(kt + 1) * P],
                            wv[:, i, :], start=(i == 0), stop=(i == 7),
                        )
                    return op

                for i in range(8):
                    ops.append(mk_mm(i))

                def fin():
                    # bias folded into the eviction (saves a K=1 matmul)
                    with nc.allow_low_precision(reason="v eviction cast"):
                        nc.vector.tensor_add(
                            v_sb[:, kt, :, 0:64], st["p"][:], bvb[:])
                ops.append(fin)
                return ops

            def proj_chunk_ops(w, br, dst, c2, pool, nm):
                """qT/kT chunk c2 (of 1024): 18 MMs + bias + evict."""
                st = {}
                ops = []

                def mk_mm(i, m):
                    def op():
                        if i == 0 and m == 0:
                            st["p"] = pool.tile([P, 1024], f32, tag="sc",
                                                name=f"pp{nm}_{c2}")
                        nc.tensor.matmul(
                            st["p"][:, m * 512:(m + 1) * 512],
                            w[:, i, :],
                            xT[:, i, c2 * 1024 + m * 512:c2 * 1024 + (m + 1) * 512],
                            start=(i == 0), stop=(i == 7),
                        )
                    return op

                for i in range(8):
                    for m in range(2):
                        ops.append(mk_mm(i, m))

                def fin():
                    # bias folded into the eviction (saves 2 K=1 matmuls)
                    with nc.allow_low_precision(reason="qk eviction cast"):
                        nc.vector.tensor_scalar_add(
                            dst[:, c2 * 1024:(c2 + 1) * 1024], st["p"][:], br)
                ops.append(fin)
                return ops

            def load_pair_weights(jt):
                js = slice(jt * P, (jt + 1) * P)
                wq = wpool.tile([P, 8, P], f16, tag="wq", name=f"wq{jt}")
                wk = wpool.tile([P, 8, P], f16, tag="wk", name=f"wk{jt}")
                for i in range(8):
                    nc.sync.dma_start(wq[:, i, :], wq_d[i * P:(i + 1) * P, js])
                    nc.sync.dma_start(wk[:, i, :], wk_d[i * P:(i + 1) * P, js])
                bqr = wpool.tile([P, 1], f32, tag="bqr", name=f"bqr{jt}")
                bkr = wpool.tile([P, 1], f32, tag="bkr", name=f"bkr{jt}")
                nc.sync.dma_start(bqr[:], bq_d[jt])
                nc.sync.dma_start(bkr[:], bk_d[jt])
                wo = wopool.tile([P, E], bf16, tag="wo", name=f"wo{jt}")
                nc.sync.dma_start(wo[:], wo_d[js, :])
                return wq, wk, bqr, bkr, wo

            def proj_pair_ops(jt, wq, wk, bqr, bkr, pool):
                qT = qkpool.tile([P, S], f16, tag="qT", name=f"qT{jt}")
                kT = qkpool.tile([P, S], f16, tag="kT", name=f"kT{jt}")
                ops = []
                for w, br, dst, nm in ((wq, bqr, qT, f"q{jt}"),
                                       (wk, bkr, kT, f"k{jt}")):
                    for c2 in range(2):
                        ops.extend(proj_chunk_ops(w, br, dst, c2, pool, nm))
                return qT, kT, ops

            def proj_half_ops(w, br, dst, c4, pool, nm):
                """One 512-col chunk (c4 of 4) of a projection (9 ops)."""
                st = {}
                ops = []

                def mk(i):
                    def op():
                        if i == 0:
                            st["p"] = pool.tile([P, 512], f32, tag="sc",
                                                name=f"ph{nm}_{c4}")
                        nc.tensor.matmul(
                            st["p"][:], w[:, i, :],
                            xT[:, i, c4 * 512:(c4 + 1) * 512],
                            start=(i == 0), stop=(i == 7),
                        )
                    return op

                for i in range(8):
                    ops.append(mk(i))

                def fin():
                    with nc.allow_low_precision(reason="qk eviction cast"):
                        nc.vector.tensor_scalar_add(
                            dst[:, c4 * 512:(c4 + 1) * 512], st["p"][:], br)
                ops.append(fin)
                return ops

            def proj_pair0_split(jt, wq, wk, bqr, bkr, pool):
                """Pair 0: emit q-chunk0 + k-chunk0 upfront; defer the second
                k chunk (as two 512-col chunks, needed from k-tile 8 / 12)
                and q-chunk1 (needed from q-chunk 2) into the filler stream
                so exp starts earlier."""
                qT = qkpool.tile([P, S], f16, tag="qT", name=f"qT{jt}")
                kT = qkpool.tile([P, S], f16, tag="kT", name=f"kT{jt}")
                pre, deferred = [], []
                pre.extend(proj_chunk_ops(wq, bqr, qT, 0, pool, f"q{jt}"))
                pre.extend(proj_chunk_ops(wk, bkr, kT, 0, pool, f"k{jt}"))
                for c4 in (2, 3):
                    deferred.extend(proj_half_ops(wk, bkr, kT, c4, pool,
                                                  f"k{jt}"))
                deferred.extend(proj_chunk_ops(wq, bqr, qT, 1, pool, f"q{jt}"))
                return qT, kT, pre, deferred

            def norm_ops(jt, qc, h2, pvc, den, outhT):
                """Deferred normalization for one (pair, q-chunk, head):
                recip (DVE) -> broadcast (GpSimd) -> mul (DVE).
                den is the denominator row pre-copied to partition 0
                (reciprocal_approx_fast is broken for inputs at partition
                base != 0 -- verified on HW)."""
                qs = slice(qc * 512, (qc + 1) * 512)
                hb = h2 * 64

                def op():
                    rc = norm.tile([1, 512], f32, tag="rc",
                                   name=f"rc{jt}_{qc}_{h2}")
                    nc.vector.reciprocal_approx_fast(rc[:], den[:])
                    bc = norm.tile([64, 512], f32, tag="bc",
                                   name=f"bc{jt}_{qc}_{h2}")
                    nc.gpsimd.partition_broadcast(bc[:], rc[:], channels=64)
                    nc.vector.tensor_mul(
                        outhT[hb:hb + 64, qs], pvc[0:64, :], bc[:])
                return [op]

            # ---- upfront: V projection (kt 0-13) + pair-0 q0/k projections;
            # V kt 14-15 and q-chunk1 run as early attention fillers ---------
            pw = {0: load_pair_weights(0)}
            for kt in range(12):
                for op in v_chunk_ops(kt, ps):
                    op()
            for kt in (12, 13):
                if kt == 13:
                    for op in v_chunk_ops(kt, ps):
                        op()
            qk = {}
            qT0, kT0, pre0, defq0 = proj_pair0_split(0, *pw[0][:4], ps)
            for op in pre0:
                op()
            # deferral order chosen so every chunk's eviction lands before
            # its first consumer at budget-3 consumption: k-half(8-11)@3,
            # v12@6, k-half(12-15)@9, v14@12, v15@15, q-chunk1@21
            deferred0 = defq0[:9]
            deferred0.extend(v_chunk_ops(12, ps))
            deferred0.extend(defq0[9:18])
            for kt in (14, 15):
                deferred0.extend(v_chunk_ops(kt, ps))
            deferred0.extend(defq0[18:])
            qk[0] = (qT0, kT0)

            # ---- attention per pair, with filler interleave -----------------
            outhT_tiles = {}
            fillers = list(reversed(deferred0))   # popped from the END
            for jt in range(NPAIR):
                qT, kT = qk[jt]
                wo_cur = pw[jt][4]

                # queue next pair's projections as fillers (due this pair)
                new_fill = []
                if jt + 1 < NPAIR:
                    pw[jt + 1] = load_pair_weights(jt + 1)
                    qTn, kTn, opsn = proj_pair_ops(jt + 1, *pw[jt + 1][:4], ps)
                    new_fill.extend(opsn)
                    qk[jt + 1] = (qTn, kTn)
                # prepend: leftovers (incl. previous pair's norm) pop first
                fillers = list(reversed(new_fill)) + fillers

                outhT = ohpool.tile([P, S], bf16, tag="outhT", name=f"oh{jt}")
                n_iters = 4 * 16
                it = 0
                for qc in range(4):
                    qs = slice(qc * 512, (qc + 1) * 512)
                    pvA = pvps.tile([65, 512], f32, tag="pvA",
                                    name=f"pvA{jt}_{qc}")
                    pvB = pvps.tile([65, 512], f32, tag="pvB",
                                    name=f"pvB{jt}_{qc}")
                    def emit_pv(k_t, et):
                        for h2, pv in ((0, pvA), (1, pvB)):
                            h = jt * 2 + h2
                            nc.tensor.matmul(
                                pv[:],
                                v_sb[:, k_t, h, :],
                                et[:, h2 * 512:(h2 + 1) * 512],
                                start=(k_t == 0), stop=(k_t == 15),
                            )

                    pend = None  # software-pipeline PV by one iteration so
                    # the in-order PE queue never head-of-line blocks on exp
                    for k_t in range(16):
                        sc = ps.tile([P, 1024], f32, tag="sc")
                        for h2 in range(2):
                            hb = h2 * 64
                            nc.tensor.matmul(
                                sc[:, h2 * 512:(h2 + 1) * 512],
                                kT[hb:hb + 64, k_t * P:(k_t + 1) * P],
                                qT[hb:hb + 64, qs],
                                start=True, stop=True,
                            )
                        et = att.tile([P, 1024], f32r, tag="exp")
                        nc.scalar.activation(
                            out=et[:], in_=sc[:], func=Exp,
                            bias=neg_c[:], scale=INV_SCALE,
                        )
                        if pend is not None:
                            emit_pv(*pend)
                        pend = (k_t, et)
                        # filler interleave: keep PE fed without starving ACT
                        it += 1
                        remaining = n_iters - it
                        nf = len(fillers)
                        budget = 3 if nf > remaining * 3 // 2 else (
                            2 if nf > remaining else (1 if nf else 0))
                        for _ in range(budget):
                            if fillers:
                                fillers.pop()()
                    emit_pv(*pend)
                    # evict pv promptly (frees the PSUM gen for qc+2);
                    # normalization itself runs later as filler ops
                    for h2, pv in ((0, pvA), (1, pvB)):
                        pvc = norm.tile([64, 512], f32, tag=f"pvc{h2}",
                                        name=f"pvc{jt}_{qc}_{h2}")
                        nc.vector.tensor_copy(pvc[:], pv[0:64, :])
                        dc = norm.tile([1, 512], f32, tag=f"den{h2}",
                                       name=f"dc{jt}_{qc}_{h2}")
                        nc.vector.tensor_copy(dc[:], pv[64:65, :])
                        # append at the END (= popped next): norm must run
                        # promptly so pvc pool gens recycle
                        fillers.extend(reversed(norm_ops(jt, qc, h2, pvc,
                                                         dc, outhT)))
                outhT_tiles[jt] = (outhT, wo_cur)

            # drain leftover fillers (last pair's norm etc.)
            while fillers:
                fillers.pop()()

            # tail: out-projection, PSUM-accumulated across all 4 pairs
            for qt in range(16):
                yp = ps.tile([P, 1024], f32, tag="sc", name=f"yps{qt}")
                for jt in range(NPAIR):
                    oprev, woprev = outhT_tiles[jt]
                    for e in range(2):
                        nc.tensor.matmul(
                            yp[:, e * 512:(e + 1) * 512],
                            oprev[:, qt * P:(qt + 1) * P],
                            woprev[:, e * 512:(e + 1) * 512],
                            start=(jt == 0), stop=(jt == NPAIR - 1),
                        )
                ysb = yout.tile([P, E], bf16, tag="ysb", name=f"ysb{qt}")
                for e in range(2):
                    es = slice(e * 512, (e + 1) * 512)
                    nc.vector.tensor_copy(ysb[:, es], yp[:, es])
                    nc.sync.dma_start(y_d[qt * P:(qt + 1) * P, es], ysb[:, es])

    nc.compile()
    return nc


def _get_nc():
    global _BUILT
    if _BUILT is None:
        _BUILT = _build()
    return _BUILT


def _prep_core_inputs(x, Wq, bq, Wk, bk, Wv, bv, Wo, g, b):
    gs = g * 512
    xT = np.ascontiguousarray(x[b].T.astype(np.float16))
    wq = np.ascontiguousarray(Wq[:, gs:gs + 512].astype(np.float16))
    wk = np.ascontiguousarray(Wk[:, gs:gs + 512].astype(np.float16))
    bqs = np.ascontiguousarray(bq[gs:gs + 512].astype(np.float32).reshape(4, P, 1))
    bks = np.ascontiguousarray(bk[gs:gs + 512].astype(np.float32).reshape(4, P, 1))
    wv = np.ascontiguousarray(Wv[:, gs:gs + 512].astype(np.float16))
    bva = np.ascontiguousarray(bv[gs:gs + 512].astype(np.float32).reshape(1, 512))
    wo = np.ascontiguousarray(Wo[gs:gs + 512, :].astype('bfloat16'))
    return {
        "xT": xT, "wq": wq, "wk": wk, "bq": bqs, "bk": bks,
        "wv": wv, "bv": bva, "wo": wo,
    }


def kernel(x, Wq, bq, Wk, bk, Wv, bv, Wo, bo):
    from concourse.bass_utils import run_bass_kernel_spmd

    x = np.asarray(x)
    B = x.shape[0]
    nc = _get_nc()
    in_maps = []
    for c in range(8):
        g, b = c // 4, c % 4
        in_maps.append(
            _prep_core_inputs(x, np.asarray(Wq), np.asarray(bq), np.asarray(Wk),
                              np.asarray(bk), np.asarray(Wv), np.asarray(bv),
                              np.asarray(Wo), g, b)
        )
    res = run_bass_kernel_spmd(nc, in_maps, list(range(8)))
    y = np.zeros((B, S, E), np.float32)
    bo = np.asarray(bo, dtype=np.float32)
    for c in range(8):
        b = c % 4
        y[b] += np.asarray(res.results[c]["y"]).astype(np.float32)
    y += bo
    return y


# revision 21
# speedup vs baseline: 1.0186x; 1.0186x over previous
"""Trainium2 Bass kernel for nn_MultiHeadAttention_60851096649901.

Sharding: 8 cores = 4 batches x 2 head-groups (8 heads each).
Each core computes its batch's attention for its 8 heads plus the partial
out-projection; host sums the two head-group partials and adds bo.

v4 structure (823us baseline -> 553us v2 -> this):
 - pv PSUM double-buffered + normalization deferred into the filler stream
   (killed the HAM re-throttle that kept the PE at 1.2 GHz 68% of the
   time); reciprocal_approx_fast (input must be at partition 0!) + GpSimd
   partition_broadcast.
 - fp16 score path (x, Wq/Wk, qT/kT): FWL weight loads, half DMA. bf16
   q/k fails the 2e-2 tolerance (exp(8s) amplifies); et/v stay f32r (bf16
   ACT output measured +20% slower; f32r cannot col-tile so the M=65
   ones-column PV is the fastest correct form).
 - V projection N=512 (ones columns memset once, not projected).
 - out-projection accumulated across head-pairs in PSUM at the tail: one
   [S,E] bf16 output per core.

Per-core math:
  qT/kT = (Wg.T @ x.T + b)        [128, 2048] per pair (d-major, fp16)
  v     = x @ Wv + bv             [2048, 8, 65] f32r (col 64 = 1.0)
  per (pair, q-chunk 512, k-tile 128):
    scoresT[k, qA|qB] via two concurrent row-group matmuls (K=64, fp16)
    p = exp(8*scores - 100) in ONE [128,1024] ACT instr -> et f32r
    pv_h[65, q] += v_h.T @ p_h  (row 64 = softmax denominator)
  normalization (deferred filler): outhT = pv[0:64] * bcast(recip(pv[64]))
  tail: y[q, :] = sum_pairs outhT_pair.T @ Wo_pair  (PSUM-accumulated)
"""

import numpy as np

S = 2048
E = 1024
D = 64
P = 128
HCORE = 8          # heads per core
NPAIR = 4          # head-pairs per core
C_OFF = 100.0      # softmax constant offset (exp(8*s - C))
INV_SCALE = 8.0    # sqrt(head_dim)

_BUILT = None


def _build():
    import concourse.bass as bass
    import concourse.tile as tile
    from concourse import bacc, mybir

    f32 = mybir.dt.float32
    f32r = mybir.dt.float32r
    bf16 = mybir.dt.bfloat16
    f16 = mybir.dt.float16
    Exp = mybir.ActivationFunctionType.Exp

    nc = bacc.Bacc("TRN2", target_bir_lowering=False, debug=False, num_devices=8)

    xT_d = nc.dram_tensor("xT", [E, S], f16, kind="ExternalInput")
    wq_d = nc.dram_tensor("wq", [E, 512], f16, kind="ExternalInput")
    wk_d = nc.dram_tensor("wk", [E, 512], f16, kind="ExternalInput")
    bq_d = nc.dram_tensor("bq", [4, P, 1], f32, kind="ExternalInput")
    bk_d = nc.dram_tensor("bk", [4, P, 1], f32, kind="ExternalInput")
    wv_d = nc.dram_tensor("wv", [E, 512], f16, kind="ExternalInput")
    bv_d = nc.dram_tensor("bv", [1, 512], f32, kind="ExternalInput")
    wo_d = nc.dram_tensor("wo", [512, E], bf16, kind="ExternalInput")
    y_d = nc.dram_tensor("y", [S, E], bf16, kind="ExternalOutput")

    with tile.TileContext(nc) as tc:
        with (
            tc.tile_pool(name="persist", bufs=1) as persist,
            tc.tile_pool(name="wpool", bufs=2) as wpool,
            tc.tile_pool(name="wopool", bufs=4) as wopool,
            tc.tile_pool(name="qk", bufs=2) as qkpool,
            tc.tile_pool(name="att", bufs=4) as att,
            tc.tile_pool(name="norm", bufs=3) as norm,
            tc.tile_pool(name="oh", bufs=4) as ohpool,
            tc.tile_pool(name="yout", bufs=2) as yout,
            tc.tile_pool(name="ps", bufs=2, space="PSUM") as ps,      # scores+fill
            tc.tile_pool(name="pvps", bufs=2, space="PSUM") as pvps,  # pv + den
        ):
            # ---- persistent loads -------------------------------------------
            xT = persist.tile([P, 8, S], f16, tag="xT")  # [i-part, i-tile, q]
            # v layout [tok-part, kt, head, 65]: col 64 of each head is the
            # constant ones column (softmax denominator via the PV matmul);
            # memset once, the V projection only writes cols 0-63
            v_sb = persist.tile([P, 16, HCORE, 65], f32r, tag="v_sb")
            for h in range(HCORE):
                nc.vector.memset(v_sb[:, :, h, 64:65].bitcast(f32), 1.0)

            neg_c = persist.tile([P, 1], f32, tag="neg_c")
            nc.vector.memset(neg_c[:], -C_OFF)
            # warm up the exp table-set (~2.7us) during the preamble
            warm = persist.tile([P, 1], f32, tag="warm")
            nc.scalar.activation(out=warm[:], in_=neg_c[:], func=Exp,
                                 bias=neg_c[:], scale=1.0)

            bvf = persist.tile([1, 512], f32, tag="bvf")
            nc.sync.dma_start(bvf[:], bv_d[:])
            bvb = persist.tile([P, 512], f32, tag="bvb")
            nc.gpsimd.partition_broadcast(bvb[:], bvf[:], channels=P)

            wv = persist.tile([P, 8, 512], f16, tag="wv")
            # interleave wv with the first xT chunk so the V projection can
            # start as soon as both land
            for i in range(8):
                nc.sync.dma_start(
                    xT[:, i, 0:512], xT_d[i * P:(i + 1) * P, 0:512])
                nc.sync.dma_start(wv[:, i, :], wv_d[i * P:(i + 1) * P, :])
            for cc in range(1, 4):
                for i in range(8):
                    nc.sync.dma_start(
                        xT[:, i, cc * 512:(cc + 1) * 512],
                        xT_d[i * P:(i + 1) * P, cc * 512:(cc + 1) * 512],
                    )

            # ---- op generators (emitted upfront or as attention fillers) ----
            def v_chunk_ops(kt, pool):
                """v[:, kt, :, 0:64] = x @ Wv + bv (9 MMs + evict)."""
                st = {}
                ops = []

                def mk_mm(i):
                    def op():
                        if i == 0:
                            st["p"] = pool.tile([P, 512], f32, tag="sc",
                                                name=f"vps{kt}")
                        nc.tensor.matmul(
                            st["p"][:], xT[:, i, kt * P:(kt + 1) * P],
                            wv[:, i, :], start=(i == 0), stop=(i == 7),
                        )
                    return op

                for i in range(8):
                    ops.append(mk_mm(i))

                def fin():
                    # bias folded into the eviction (saves a K=1 matmul)
                    with nc.allow_low_precision(reason="v eviction cast"):
                        nc.vector.tensor_add(
                            v_sb[:, kt, :, 0:64], st["p"][:], bvb[:])
                ops.append(fin)
                return ops

            def proj_chunk_ops(w, br, dst, c2, pool, nm):
                """qT/kT chunk c2 (of 1024): 18 MMs + bias + evict."""
                st = {}
                ops = []

                def mk_mm(i, m):
                    def op():
                        if i == 0 and m == 0:
                            st["p"] = pool.tile([P, 1024], f32, tag="sc",
                                                name=f"pp{nm}_{c2}")
                        nc.tensor.matmul(
                            st["p"][:, m * 512:(m + 1) * 512],
                            w[:, i, :],
                            xT[:, i, c2 * 1024 + m * 512:c2 * 1024 + (m + 1) * 512],
                            start=(i == 0), stop=(i == 7),
                        )
                    return op

                for i in range(8):
                    for m in range(2):
                        ops.append(mk_mm(i, m))

                def fin():
                    # bias folded into the eviction (saves 2 K=1 matmuls)
                    with nc.allow_low_precision(reason="qk eviction cast"):
                        nc.vector.tensor_scalar_add(
                            dst[:, c2 * 1024:(c2 + 1) * 1024], st["p"][:], br)
                ops.append(fin)
                return ops

            def load_pair_weights(jt):
                js = slice(jt * P, (jt + 1) * P)
                wq = wpool.tile([P, 8, P], f16, tag="wq", name=f"wq{jt}")
                wk = wpool.tile([P, 8, P], f16, tag="wk", name=f"wk{jt}")
                for i in range(8):
                    nc.sync.dma_start(wq[:, i, :], wq_d[i * P:(i + 1) * P, js])
                    nc.sync.dma_start(wk[:, i, :], wk_d[i * P:(i + 1) * P, js])
                bqr = wpool.tile([P, 1], f32, tag="bqr", name=f"bqr{jt}")
                bkr = wpool.tile([P, 1], f32, tag="bkr", name=f"bkr{jt}")
                nc.sync.dma_start(bqr[:], bq_d[jt])
                nc.sync.dma_start(bkr[:], bk_d[jt])
                wo = wopool.tile([P, E], bf16, tag="wo", name=f"wo{jt}")
                nc.sync.dma_start(wo[:], wo_d[js, :])
                return wq, wk, bqr, bkr, wo

            def proj_pair_ops(jt, wq, wk, bqr, bkr, pool):
                qT = qkpool.tile([P, S], f16, tag="qT", name=f"qT{jt}")
                kT = qkpool.tile([P, S], f16, tag="kT", name=f"kT{jt}")
                ops = []
                for w, br, dst, nm in ((wq, bqr, qT, f"q{jt}"),
                                       (wk, bkr, kT, f"k{jt}")):
                    for c2 in range(2):
                        ops.extend(proj_chunk_ops(w, br, dst, c2, pool, nm))
                return qT, kT, ops

            def proj_half_ops(w, br, dst, c4, pool, nm):
                """One 512-col chunk (c4 of 4) of a projection (9 ops)."""
                st = {}
                ops = []

                def mk(i):
                    def op():
                        if i == 0:
                            st["p"] = pool.tile([P, 512], f32, tag="sc",
                                                name=f"ph{nm}_{c4}")
                        nc.tensor.matmul(
                            st["p"][:], w[:, i, :],
                            xT[:, i, c4 * 512:(c4 + 1) * 512],
                            start=(i == 0), stop=(i == 7),
                        )
                    return op

                for i in range(8):
                    ops.append(mk(i))

                def fin():
                    with nc.allow_low_precision(reason="qk eviction cast"):
                        nc.vector.tensor_scalar_add(
                            dst[:, c4 * 512:(c4 + 1) * 512], st["p"][:], br)
                ops.append(fin)
                return ops

            def proj_pair0_split(jt, wq, wk, bqr, bkr, pool):
                """Pair 0: emit q-chunk0 + k-chunk0 upfront; defer the second
                k chunk (as two 512-col chunks, needed from k-tile 8 / 12)
                and q-chunk1 (needed from q-chunk 2) into the filler stream
                so exp starts earlier."""
                qT = qkpool.tile([P, S], f16, tag="qT", name=f"qT{jt}")
                kT = qkpool.tile([P, S], f16, tag="kT", name=f"kT{jt}")
                pre, deferred = [], []
                pre.extend(proj_chunk_ops(wq, bqr, qT, 0, pool, f"q{jt}"))
                pre.extend(proj_chunk_ops(wk, bkr, kT, 0, pool, f"k{jt}"))
                for c4 in (2, 3):
                    deferred.extend(proj_half_ops(wk, bkr, kT, c4, pool,
                                                  f"k{jt}"))
                deferred.extend(proj_chunk_ops(wq, bqr, qT, 1, pool, f"q{jt}"))
                return qT, kT, pre, deferred

            def norm_ops(jt, qc, h2, pvc, den, outhT):
                """Deferred normalization for one (pair, q-chunk, head):
                recip (DVE) -> broadcast (GpSimd) -> mul (DVE).
                den is the denominator row pre-copied to partition 0
                (reciprocal_approx_fast is broken for inputs at partition
                base != 0 -- verified on HW)."""
                qs = slice(qc * 512, (qc + 1) * 512)
                hb = h2 * 64

                def op():
                    rc = norm.tile([1, 512], f32, tag="rc",
                                   name=f"rc{jt}_{qc}_{h2}")
                    nc.vector.reciprocal_approx_fast(rc[:], den[:])
                    bc = norm.tile([64, 512], f32, tag="bc",
                                   name=f"bc{jt}_{qc}_{h2}")
                    nc.gpsimd.partition_broadcast(bc[:], rc[:], channels=64)
                    nc.vector.tensor_mul(
                        outhT[hb:hb + 64, qs], pvc[0:64, :], bc[:])
                return [op]

            # ---- upfront: V projection (kt 0-13) + pair-0 q0/k projections;
            # V kt 14-15 and q-chunk1 run as early attention fillers ---------
            pw = {0: load_pair_weights(0)}
            for kt in range(14):
                for op in v_chunk_ops(kt, ps):
                    op()
            qk = {}
            qT0, kT0, pre0, defq0 = proj_pair0_split(0, *pw[0][:4], ps)
            for op in pre0:
                op()
            # order matters: k-halves land before k-tile 8/12, V before
            # k-tiles 14/15, q-chunk1 before q-chunk 2 (budget-3 early)
            deferred0 = defq0[:18]
            for kt in (14, 15):
                deferred0.extend(v_chunk_ops(kt, ps))
            deferred0.extend(defq0[18:])
            qk[0] = (qT0, kT0)

            # ---- attention per pair, with filler interleave -----------------
            outhT_tiles = {}
            fillers = list(reversed(deferred0))   # popped from the END
            for jt in range(NPAIR):
                qT, kT = qk[jt]
                wo_cur = pw[jt][4]

                # queue next pair's projections as fillers (due this pair)
                new_fill = []
                if jt + 1 < NPAIR:
                    pw[jt + 1] = load_pair_weights(jt + 1)
                    qTn, kTn, opsn = proj_pair_ops(jt + 1, *pw[jt + 1][:4], ps)
                    new_fill.extend(opsn)
                    qk[jt + 1] = (qTn, kTn)
                # prepend: leftovers (incl. previous pair's norm) pop first
                fillers = list(reversed(new_fill)) + fillers

                outhT = ohpool.tile([P, S], bf16, tag="outhT", name=f"oh{jt}")
                n_iters = 4 * 16
                it = 0
                for qc in range(4):
                    qs = slice(qc * 512, (qc + 1) * 512)
                    pvA = pvps.tile([65, 512], f32, tag="pvA",
                                    name=f"pvA{jt}_{qc}")
                    pvB = pvps.tile([65, 512], f32, tag="pvB",
                                    name=f"pvB{jt}_{qc}")
                    def emit_pv(k_t, et):
                        for h2, pv in ((0, pvA), (1, pvB)):
                            h = jt * 2 + h2
                            nc.tensor.matmul(
                                pv[:],
                                v_sb[:, k_t, h, :],
                                et[:, h2 * 512:(h2 + 1) * 512],
                                start=(k_t == 0), stop=(k_t == 15),
                            )

                    pend = None  # software-pipeline PV by one iteration so
                    # the in-order PE queue never head-of-line blocks on exp
                    for k_t in range(16):
                        sc = ps.tile([P, 1024], f32, tag="sc")
                        for h2 in range(2):
                            hb = h2 * 64
                            nc.tensor.matmul(
                                sc[:, h2 * 512:(h2 + 1) * 512],
                                kT[hb:hb + 64, k_t * P:(k_t + 1) * P],
                                qT[hb:hb + 64, qs],
                                start=True, stop=True,
                            )
                        et = att.tile([P, 1024], f32r, tag="exp")
                        nc.scalar.activation(
                            out=et[:], in_=sc[:], func=Exp,
                            bias=neg_c[:], scale=INV_SCALE,
                        )
                        if pend is not None:
                            emit_pv(*pend)
                        pend = (k_t, et)
                        # filler interleave: keep PE fed without starving ACT
                        it += 1
                        remaining = n_iters - it
                        nf = len(fillers)
                        budget = 3 if nf > remaining * 3 // 2 else (
                            2 if nf > remaining else (1 if nf else 0))
                        for _ in range(budget):
                            if fillers:
                                fillers.pop()()
                    emit_pv(*pend)
                    # evict pv promptly (frees the PSUM gen for qc+2);
                    # normalization itself runs later as filler ops
                    for h2, pv in ((0, pvA), (1, pvB)):
                        pvc = norm.tile([64, 512], f32, tag=f"pvc{h2}",
                                        name=f"pvc{jt}_{qc}_{h2}")
                        nc.vector.tensor_copy(pvc[:], pv[0:64, :])
                        dc = norm.tile([1, 512], f32, tag=f"den{h2}",
                                       name=f"dc{jt}_{qc}_{h2}")
                        nc.vector.tensor_copy(dc[:], pv[64:65, :])
                        # append at the END (= popped next): norm must run
                        # promptly so pvc pool gens recycle
                        fillers.extend(reversed(norm_ops(jt, qc, h2, pvc,
                                                         dc, outhT)))
                outhT_tiles[jt] = (outhT, wo_cur)

            # drain leftover fillers (last pair's norm etc.)
            while fillers:
                fillers.pop()()

            # tail: out-projection, PSUM-accumulated across all 4 pairs
            for qt in range(16):
                yp = ps.tile([P, 1024], f32, tag="sc", name=f"yps{qt}")
                for jt in range(NPAIR):
                    oprev, woprev = outhT_tiles[jt]
                    for e in range(2):
                        nc.tensor.matmul(
                            yp[:, e * 512:(e + 1) * 512],
                            oprev[:, qt * P:(qt + 1) * P],
                            woprev[:, e * 512:(e + 1) * 512],
                            start=(jt == 0), stop=(jt == NPAIR - 1),
                        )
                ysb = yout.tile([P, E], bf16, tag="ysb", name=f"ysb{qt}")
                nc.vector.tensor_copy(ysb[:], yp[:])
                nc.sync.dma_start(y_d[qt * P:(qt + 1) * P, :], ysb[:])

    nc.compile()
    return nc


def _get_nc():
    global _BUILT
    if _BUILT is None:
        _BUILT = _build()
    return _BUILT


def _prep_core_inputs(x, Wq, bq, Wk, bk, Wv, bv, Wo, g, b):
    gs = g * 512
    xT = np.ascontiguousarray(x[b].T.astype(np.float16))
    wq = np.ascontiguousarray(Wq[:, gs:gs + 512].astype(np.float16))
    wk = np.ascontiguousarray(Wk[:, gs:gs + 512].astype(np.float16))
    bqs = np.ascontiguousarray(bq[gs:gs + 512].astype(np.float32).reshape(4, P, 1))
    bks = np.ascontiguousarray(bk[gs:gs + 512].astype(np.float32).reshape(4, P, 1))
    wv = np.ascontiguousarray(Wv[:, gs:gs + 512].astype(np.float16))
    bva = np.ascontiguousarray(bv[gs:gs + 512].astype(np.float32).reshape(1, 512))
    wo = np.ascontiguousarray(Wo[gs:gs + 512, :].astype('bfloat16'))
    return {
        "xT": xT, "wq": wq, "wk": wk, "bq": bqs, "bk": bks,
        "wv": wv, "bv": bva, "wo": wo,
    }


def kernel(x, Wq, bq, Wk, bk, Wv, bv, Wo, bo):
    from concourse.bass_utils import run_bass_kernel_spmd

    x = np.asarray(x)
    B = x.shape[0]
    nc = _get_nc()
    in_maps = []
    for c in range(8):
        g, b = c // 4, c % 4
        in_maps.append(
            _prep_core_inputs(x, np.asarray(Wq), np.asarray(bq), np.asarray(Wk),
                              np.asarray(bk), np.asarray(Wv), np.asarray(bv),
                              np.asarray(Wo), g, b)
        )
    res = run_bass_kernel_spmd(nc, in_maps, list(range(8)))
    y = np.zeros((B, S, E), np.float32)
    bo = np.asarray(bo, dtype=np.float32)
    for c in range(8):
        b = c % 4
        y[b] += np.asarray(res.results[c]["y"]).astype(np.float32)
    y += bo
    return y
